# revision 5
# baseline (speedup 1.0000x reference)
"""Trainium2 Bass kernel for the CopyGenerator problem.

Computation (see reference):
  context = einsum(bts,sbh->tbh);  p = sigmoid(context @ W_prob + b_prob)
  out = log(p * softmax(X @ W_gen + b_gen) + (1-p) * softmax(scatter_add(attn)))

Strategy: shard the vocab dimension V across the 8 NeuronCores (tensor
parallel).  Each core computes its [T*B, V/8] slice of the gen logits with a
bf16 matmul, exponentiates it into an SBUF-resident slab (no DRAM round
trip), and handles the sparse copy-scatter by adding F*(exp(cv)-1) onto the
exp-slab at the touched vocab columns via a GPSIMD scatter_add (pair-indexed,
d=2).  Softmax normalizers (and the per-batch copy gate p, computed
data-parallel, one batch column per core) are exchanged with a single tiny
AllGather.  The final pass is out = ln(A * eg + C) streamed straight to the
output DRAM slice.

Row layout is b-major (r = b*T + t) so every 64-partition half of a row-tile
shares one batch index -> the scatter/cv machinery needs no per-partition
index variation beyond what scatter_add supports.

Host-side work is limited to dtype casts, layout permutations/slices and
int32 index-table construction from src_full (no float arithmetic).
"""

import numpy as np
import ml_dtypes

V = 50257
H = 512
S = 512
B = 8
T = 64
R = B * T            # 512 rows, b-major: r = b*T + t
NCORES = 8
VC = 6283            # per-core vocab shard (8*6283 = 50264 >= V)
WSLAB = VC + 1       # 6284 (even) slab width; col 6283 is a junk/pad column
PAIRS = WSLAB // 2   # 3142
MT = 4               # row tiles of 128
NT = 13              # vocab tiles per core: 12*512 + 140
NTW = [512] * 12 + [WSLAB - 512 * 12]       # compute widths (140 last)
NTW_REAL = [512] * 12 + [VC - 512 * 12]     # written widths (139 last)
BF16 = ml_dtypes.bfloat16

_COMPILED = {}
LAST_RESULTS = None  # BassKernelResults of the most recent run (for test.py)
TRACE = False        # test.py can flip this to capture an NTFF profile


def _build_program(E_pad):
    import concourse.bacc as bacc
    import concourse.tile as tile
    import concourse.mybir as mybir

    dt = mybir.dt
    AF = mybir.ActivationFunctionType
    ALU = mybir.AluOpType
    X_AX = mybir.AxisListType.X

    nc = bacc.Bacc("TRN2", target_bir_lowering=False, debug=False,
                   num_devices=NCORES)

    f32 = dt.float32
    bf16 = dt.bfloat16

    xT = nc.dram_tensor("xT", [H, R], bf16, kind="ExternalInput").ap()
    wg = nc.dram_tensor("wg", [H, WSLAB], bf16, kind="ExternalInput").ap()
    bg = nc.dram_tensor("bg", [1, WSLAB], bf16, kind="ExternalInput").ap()
    aPT = nc.dram_tensor("aPT", [S, R], bf16, kind="ExternalInput").ap()
    ohe = nc.dram_tensor("ohe", [S, B * E_pad], bf16, kind="ExternalInput").ap()
    oho = nc.dram_tensor("oho", [S, B * E_pad], bf16, kind="ExternalInput").ap()
    mT = nc.dram_tensor("mT", [H, S], f32, kind="ExternalInput").ap()
    aTk = nc.dram_tensor("aTk", [S, T], f32, kind="ExternalInput").ap()
    wp = nc.dram_tensor("wp", [H, 1], f32, kind="ExternalInput").ap()
    bp = nc.dram_tensor("bp", [1, 1], f32, kind="ExternalInput").ap()
    scidx = nc.dram_tensor("scidx", [128, MT, E_pad // 16], dt.int16,
                           kind="ExternalInput").ap()
    out = nc.dram_tensor("out", [R, VC], f32, kind="ExternalOutput").ap()

    with tile.TileContext(nc) as tc:
        with (
            tc.tile_pool(name="cpool", bufs=1) as cpool,
            tc.tile_pool(name="wpool", bufs=2) as wpool,
            tc.tile_pool(name="spool", bufs=1) as spool,
            tc.tile_pool(name="apool", bufs=2) as apool,
            tc.tile_pool(name="tpool", bufs=3) as tpool,
            tc.tile_pool(name="opool", bufs=3) as opool,
            tc.tile_pool(name="psg", bufs=3, space="PSUM") as psg,
            tc.tile_pool(name="psc", bufs=1, space="PSUM") as psc,
            tc.tile_pool(name="psp", bufs=1, space="PSUM") as psp,
            tc.tile_pool(name="dram", bufs=1, space="DRAM") as dram,
        ):
            # ---------------- constant loads ----------------
            xT_sb = cpool.tile([128, 4, R], bf16)
            nc.sync.dma_start(xT_sb[:], xT.rearrange("(k p) r -> p k r", p=128))
            aPT_sb = cpool.tile([128, 4, R], bf16)
            nc.sync.dma_start(aPT_sb[:], aPT.rearrange("(k p) r -> p k r", p=128))
            ohe_sb = cpool.tile([128, 4, B * E_pad], bf16)
            nc.sync.dma_start(ohe_sb[:], ohe.rearrange("(k p) e -> p k e", p=128))
            oho_sb = cpool.tile([128, 4, B * E_pad], bf16)
            nc.sync.dma_start(oho_sb[:], oho.rearrange("(k p) e -> p k e", p=128))
            mT_sb = cpool.tile([128, 4, S], f32)
            nc.sync.dma_start(mT_sb[:], mT.rearrange("(k p) s -> p k s", p=128))
            aTk_sb = cpool.tile([128, 4, T], f32)
            nc.sync.dma_start(aTk_sb[:], aTk.rearrange("(k p) t -> p k t", p=128))
            wp_sb = cpool.tile([128, 4, 1], f32)
            nc.sync.dma_start(wp_sb[:], wp.rearrange("(k p) o -> p k o", p=128))
            bp_sb = cpool.tile([1, 1], f32)
            nc.sync.dma_start(bp_sb[:], bp[:, :])
            bg_sb = cpool.tile([1, WSLAB], bf16)
            nc.sync.dma_start(bg_sb[:], bg[:, :])
            sc_sb = cpool.tile([128, MT, E_pad // 16], dt.int16)
            nc.sync.dma_start(sc_sb[:], scidx[:, :, :])
            ones128 = cpool.tile([1, 128], bf16)
            nc.vector.memset(ones128[:], 1.0)
            ones64 = cpool.tile([1, T], f32)
            nc.vector.memset(ones64[:], 1.0)

            eg = cpool.tile([128, MT, WSLAB], bf16)       # exp(gen logits)
            cvexp = cpool.tile([128, MT, 2, E_pad], f32)  # exp(copy vals)

            accg = spool.tile([128, MT, 16], f32)
            nc.vector.memset(accg[:], 0.0)
            acce = spool.tile([128, MT], f32)
            acco = spool.tile([128, MT], f32)

            cc_in = dram.tile([1, 1088], f32)
            cc_out = dram.tile([NCORES, 1088], f32)

            # ---------------- p path (this core's batch column b = core id)
            # mv[s] = memory[s, k, :] @ W_prob
            mv_ps = psp.tile([128, 4], f32)
            for st in range(4):
                for ht in range(4):
                    nc.tensor.matmul(
                        mv_ps[:, st:st + 1],
                        lhsT=mT_sb[:, ht, st * 128:(st + 1) * 128],
                        rhs=wp_sb[:, ht, :],
                        start=(ht == 0), stop=(ht == 3))
            mv_sb = spool.tile([128, 4], f32)
            nc.scalar.copy(mv_sb[:], mv_ps[:])
            # plogit[t] = attn[k, t, :] @ mv + b_prob
            pp_ps = psp.tile([T, 1], f32)
            for st in range(4):
                nc.tensor.matmul(pp_ps[:], lhsT=aTk_sb[:, st, :],
                                 rhs=mv_sb[:, st:st + 1],
                                 start=(st == 0), stop=False)
            nc.tensor.matmul(pp_ps[:], lhsT=ones64[:], rhs=bp_sb[:],
                             start=False, stop=True)
            p64 = spool.tile([T, 1], f32)
            nc.scalar.activation(p64[:], pp_ps[:], AF.Sigmoid)
            nc.sync.dma_start(cc_in[0:1, 0:T].rearrange("o t -> t o"), p64[:])

            # ---------------- copy-value path
            # cv[r, e] = sum_s attn[b, t, s] * onehot_b[s, e], per parity
            for m in range(MT):
                cve = psc.tile([128, E_pad], f32)
                cvo = psc.tile([128, E_pad], f32)
                for half in range(2):
                    b = 2 * m + half
                    prt = slice(64 * half, 64 * half + 64)
                    col = m * 128 + 64 * half
                    for kt in range(4):
                        nc.tensor.matmul(
                            cve[prt, :],
                            lhsT=aPT_sb[:, kt, col:col + 64],
                            rhs=ohe_sb[:, kt, b * E_pad:(b + 1) * E_pad],
                            start=(kt == 0), stop=(kt == 3))
                    for kt in range(4):
                        nc.tensor.matmul(
                            cvo[prt, :],
                            lhsT=aPT_sb[:, kt, col:col + 64],
                            rhs=oho_sb[:, kt, b * E_pad:(b + 1) * E_pad],
                            start=(kt == 0), stop=(kt == 3))
                nc.scalar.activation(cvexp[:, m, 0, :], cve[:], AF.Exp,
                                     accum_out=acce[:, m:m + 1])
                nc.scalar.activation(cvexp[:, m, 1, :], cvo[:], AF.Exp,
                                     accum_out=acco[:, m:m + 1])

            # local copy-softmax partial: sum_e (exp(cv) - 1)
            zc_loc = spool.tile([128, MT], f32)
            nc.vector.tensor_add(zc_loc[:], acce[:], acco[:])
            nc.vector.tensor_scalar(zc_loc[:], zc_loc[:], -2.0 * E_pad, None,
                                    ALU.add)
            nc.sync.dma_start(
                cc_in[0:1, 576:1088].rearrange("o (m p) -> p (m o)", m=MT, p=128),
                zc_loc[:])

            # ---------------- gen matmul + exp into the slab
            for n in range(NT):
                wn = NTW[n]
                ns = 512 * n
                wg_t = wpool.tile([128, 4, wn], bf16, tag="wgt")
                nc.sync.dma_start(
                    wg_t[:], wg[:, ns:ns + wn].rearrange("(k p) w -> p k w", p=128))
                for m in range(MT):
                    gp = psg.tile([128, wn], f32, tag="gp")
                    for kt in range(4):
                        nc.tensor.matmul(gp[:],
                                         lhsT=xT_sb[:, kt, m * 128:(m + 1) * 128],
                                         rhs=wg_t[:, kt, :],
                                         start=(kt == 0), stop=False)
                    nc.tensor.matmul(gp[:], lhsT=ones128[:],
                                     rhs=bg_sb[:, ns:ns + wn],
                                     start=False, stop=True)
                    nc.scalar.activation(eg[:, m, ns:ns + wn], gp[:], AF.Exp,
                                         accum_out=accg[:, m, n:n + 1])

            zg_loc = spool.tile([128, MT], f32)
            nc.vector.reduce_sum(zg_loc[:], accg[:], axis=X_AX)
            nc.sync.dma_start(
                cc_in[0:1, 64:576].rearrange("o (m p) -> p (m o)", m=MT, p=128),
                zg_loc[:])

            # ---------------- normalizer exchange
            nc.gpsimd.collective_compute(
                "AllGather", ALU.bypass,
                replica_groups=[list(range(NCORES))],
                ins=[cc_in[:].opt()], outs=[cc_out[:].opt()])

            p_all = spool.tile([128, MT], f32)
            for m in range(MT):
                for half in range(2):
                    b = 2 * m + half
                    nc.sync.dma_start(
                        p_all[64 * half:64 * half + 64, m:m + 1],
                        cc_out[b:b + 1, 0:T].rearrange("o t -> t o"))
            zg_all = spool.tile([128, MT, NCORES], f32)
            zc_all = spool.tile([128, MT, NCORES], f32)
            for c in range(NCORES):
                nc.sync.dma_start(
                    zg_all[:, :, c:c + 1],
                    cc_out[c:c + 1, 64:576].rearrange("o (m p) -> p m o", m=MT, p=128))
                nc.sync.dma_start(
                    zc_all[:, :, c:c + 1],
                    cc_out[c:c + 1, 576:1088].rearrange("o (m p) -> p m o", m=MT, p=128))

            zg = spool.tile([128, MT], f32)
            nc.vector.reduce_sum(zg[:], zg_all[:], axis=X_AX)
            zc = spool.tile([128, MT], f32)
            nc.vector.reduce_sum(zc[:], zc_all[:], axis=X_AX)
            nc.vector.tensor_scalar(zc[:], zc[:], float(V), None, ALU.add)
            zg_i = spool.tile([128, MT], f32)
            nc.vector.reciprocal(zg_i[:], zg[:])
            zc_i = spool.tile([128, MT], f32)
            nc.vector.reciprocal(zc_i[:], zc[:])
            p_i = spool.tile([128, MT], f32)
            nc.vector.reciprocal(p_i[:], p_all[:])
            A = spool.tile([128, MT], f32)
            nc.vector.tensor_mul(A[:], p_all[:], zg_i[:])
            Cc = spool.tile([128, MT], f32)
            nc.vector.tensor_scalar(Cc[:], p_all[:], -1.0, 1.0, ALU.mult, ALU.add)
            nc.vector.tensor_mul(Cc[:], Cc[:], zc_i[:])
            F = spool.tile([128, MT], f32)
            nc.vector.tensor_mul(F[:], Cc[:], zg[:])
            nc.vector.tensor_mul(F[:], F[:], p_i[:])

            # ---------------- sparse fixup + final pass
            for m in range(MT):
                addt = apool.tile([128, E_pad, 2], bf16, tag="addt")
                nc.vector.tensor_scalar(addt[:, :, 0], cvexp[:, m, 0, :],
                                        -1.0, F[:, m:m + 1], ALU.add, ALU.mult)
                nc.vector.tensor_scalar(addt[:, :, 1], cvexp[:, m, 1, :],
                                        -1.0, F[:, m:m + 1], ALU.add, ALU.mult)
                nc.gpsimd.scatter_add(
                    eg[:, m, :].rearrange("c (q d) -> c q d", d=2),
                    sc_sb[:, m, :],
                    addt[:, :, :],
                    channels=128, num_elems=PAIRS, d=2, num_idxs=E_pad)
                for n in range(NT):
                    wn = NTW[n]
                    wr = NTW_REAL[n]
                    ns = 512 * n
                    tmp = tpool.tile([128, wn], bf16, tag="tmp")
                    nc.vector.tensor_scalar(tmp[:], eg[:, m, ns:ns + wn],
                                            A[:, m:m + 1], Cc[:, m:m + 1],
                                            ALU.mult, ALU.add)
                    ot = opool.tile([128, wn], f32, tag="ot")
                    nc.scalar.activation(ot[:], tmp[:], AF.Ln)
                    nc.sync.dma_start(out[m * 128:(m + 1) * 128, ns:ns + wr],
                                      ot[:, :wr])

    nc.compile()
    return nc


def _host_prep(inputs):
    src = np.asarray(inputs["src_full"]).astype(np.int64)          # [S, B]
    X = np.asarray(inputs["decode_output"], dtype=np.float32)      # [T, B, H]
    attn = np.asarray(inputs["decode_attn"], dtype=np.float32)     # [B, T, S]
    mem = np.asarray(inputs["memory"], dtype=np.float32)           # [S, B, H]
    Wg = np.asarray(inputs["W_gen"], dtype=np.float32)             # [H, V]
    bgv = np.asarray(inputs["b_gen"], dtype=np.float32)            # [V]
    Wp = np.asarray(inputs["W_prob"], dtype=np.float32)            # [H, 1]
    bpv = np.asarray(inputs["b_prob"], dtype=np.float32)           # [1]

    pairs_kb = {}
    for k in range(NCORES):
        lo = k * VC
        for b in range(B):
            c = src[:, b]
            selm = (c >= lo) & (c < lo + VC) & (c < V)
            pairs_kb[(k, b)] = np.unique((c[selm] - lo) // 2)
    # union pair list per (core, m-tile): one shared index list for both
    # batches of the tile -> scatter_add runs with channels=128 and all APs
    # at partition base 0 (HW ucode does not honor partition offsets).
    pairs_km = {}
    Emax = 1
    for k in range(NCORES):
        for m in range(MT):
            L = np.union1d(pairs_kb[(k, 2 * m)], pairs_kb[(k, 2 * m + 1)])
            pairs_km[(k, m)] = L
            Emax = max(Emax, len(L))
    E_pad = ((Emax + 15) // 16) * 16

    X_bm = X.transpose(1, 0, 2).reshape(R, H)                  # r = b*T + t
    xT_h = np.ascontiguousarray(X_bm.T).astype(BF16)           # [H, R]
    aPT_h = np.ascontiguousarray(attn.reshape(R, S).T).astype(BF16)  # [S, R]

    in_maps = []
    for k in range(NCORES):
        lo = k * VC
        wk = min(V - lo, VC)
        wg_h = np.zeros((H, WSLAB), np.float32)
        wg_h[:, :wk] = Wg[:, lo:lo + wk]
        bg_h = np.full((1, WSLAB), -30.0, np.float32)
        bg_h[0, :wk] = bgv[lo:lo + wk]
        ohe_h = np.zeros((S, B * E_pad), np.float32)
        oho_h = np.zeros((S, B * E_pad), np.float32)
        sc_h = np.full((128, MT, E_pad // 16), -1, np.int16)
        for b in range(B):
            L = pairs_km[(k, b // 2)]
            assert len(L) >= 1
            lpos = {int(qq): i for i, qq in enumerate(L)}
            c = src[:, b]
            selm = (c >= lo) & (c < lo + VC) & (c < V)
            for s in np.nonzero(selm)[0]:
                j = int(c[s]) - lo
                e = lpos[j // 2]
                if j % 2 == 0:
                    ohe_h[s, b * E_pad + e] = 1.0
                else:
                    oho_h[s, b * E_pad + e] = 1.0
        for m in range(MT):
            L = pairs_km[(k, m)]
            lst = np.full((E_pad,), -1, np.int64)
            lst[:len(L)] = L
            wrapped = np.zeros((16, E_pad // 16), np.int16)
            for i, val in enumerate(lst):
                wrapped[i % 16, i // 16] = val
            sc_h[:, m, :] = np.tile(wrapped, (8, 1))
        in_maps.append({
            "xT": xT_h,
            "wg": wg_h.astype(BF16),
            "bg": bg_h.astype(BF16),
            "aPT": aPT_h,
            "ohe": ohe_h.astype(BF16),
            "oho": oho_h.astype(BF16),
            "mT": np.ascontiguousarray(mem[:, k, :].T).astype(np.float32),
            "aTk": np.ascontiguousarray(attn[k].T).astype(np.float32),
            "wp": Wp.astype(np.float32),
            "bp": bpv.reshape(1, 1).astype(np.float32),
            "scidx": sc_h,
        })
    return in_maps, E_pad


def _get_program(E_pad):
    if E_pad not in _COMPILED:
        _COMPILED[E_pad] = _build_program(E_pad)
    return _COMPILED[E_pad]


def kernel(**inputs):
    global LAST_RESULTS
    from concourse import bass_utils

    in_maps, E_pad = _host_prep(inputs)
    nc = _get_program(E_pad)
    res = bass_utils.run_bass_kernel_spmd(
        nc, in_maps, core_ids=list(range(NCORES)), trace=TRACE)
    LAST_RESULTS = res
    outs = [r["out"] for r in res.results]                 # [R, VC] f32 each
    full = np.concatenate(
        [outs[k][:, :min(V - k * VC, VC)] for k in range(NCORES)], axis=1)
    return np.ascontiguousarray(
        full.reshape(B, T, V).transpose(1, 0, 2)).astype(np.float32)
